# revision 3
# baseline (speedup 1.0000x reference)
# Trainium2 Bass kernel for nn_HamEvo_56006373540016.
#
# Math: the reference integrates ds/dt = -i H s with RK4 (10 steps, 4 stages)
# where H acts only on qubits (18, 19) of a 20-qubit state — i.e. a 4x4
# complex matrix per batch element applied along the "s" axis of
# state[x, s, b] (x = 2^18 spectator index, s = 4, b = 16 batch).
# RK4 on a LINEAR ODE is exactly the degree-4 Taylor polynomial of exp(hA),
# so the whole 10-step evolution collapses to one 4x4 complex matrix per
# batch: E_b = (I + hA + (hA)^2/2 + (hA)^3/6 + (hA)^4/24)^10, A = -i G_b.
# We precompute E_b on the host in float64, realify it into an 8x8 real block
# (acting on [re(4); im(4)]), and assemble a 128x128 block-diagonal weight
# over the 16 batches. The device kernel is then a single streamed matmul:
#   Y[128, x] = W[128, 128] @ X[128, x]      (partition dim = (b, c, s))
# which reads the state once and writes it once — memory-bound.
#
# Sharding: the x axis (2^18 values) is split contiguously across 8 cores
# (zero communication; every core gets all batches and the same weight).

import numpy as np

P = 128
B = 16
S = 4
X18 = 1 << 18            # number of x values (qubits 0..17)
NCORES = 8
XC = X18 // NCORES       # 32768 x values per core
FT = 4096                # free elems per DMA tile ([128, FT] f32 = 2 MiB)
MM = 512                 # matmul free dim (one PSUM bank of fp32)

_PERM = np.array([0, 2, 1, 3])  # bit-swap of the 2-qubit index (pyqtorch order)

_NC_CACHE = {}


def _build_nc():
    """Build the Bass program (same SPMD program for all 8 cores)."""
    import concourse.mybir as mybir
    from concourse import bacc
    from concourse.tile import TileContext

    nc = bacc.Bacc(
        "TRN2", target_bir_lowering=False, debug=False, num_devices=NCORES
    )
    w = nc.dram_tensor("w", [P, P], mybir.dt.float32, kind="ExternalInput")
    x = nc.dram_tensor("x", [P, XC], mybir.dt.float32, kind="ExternalInput")
    y = nc.dram_tensor("y", [P, XC], mybir.dt.float32, kind="ExternalOutput")

    with TileContext(nc) as tc:
        with (
            tc.tile_pool(name="wp", bufs=1) as wp,
            tc.tile_pool(name="xin", bufs=3) as xin,
            tc.tile_pool(name="yout", bufs=3) as yout,
            tc.tile_pool(name="ps", bufs=8, space="PSUM") as ps,
        ):
            wt = wp.tile([P, P], mybir.dt.float32)
            nc.sync.dma_start(wt[:], w[:])
            for i in range(XC // FT):
                xt = xin.tile([P, FT], mybir.dt.float32)
                nc.sync.dma_start(xt[:], x[:, i * FT:(i + 1) * FT])
                yt = yout.tile([P, FT], mybir.dt.float32)
                for j in range(FT // MM):
                    pt = ps.tile([P, MM], mybir.dt.float32)
                    nc.tensor.matmul(
                        pt[:], wt[:], xt[:, j * MM:(j + 1) * MM]
                    )
                    nc.vector.tensor_copy(yt[:, j * MM:(j + 1) * MM], pt[:])
                nc.sync.dma_start(y[:, i * FT:(i + 1) * FT], yt[:])
    nc.compile()
    return nc


def _get_nc():
    if "nc" not in _NC_CACHE:
        _NC_CACHE["nc"] = _build_nc()
    return _NC_CACHE["nc"]


def _build_weight(H_re, H_im, t):
    """128x128 block-diag weight: per-batch realified 10-step RK4 evolution."""
    H = H_re.astype(np.float64) + 1j * H_im.astype(np.float64)  # (4,4,B)
    G = H[_PERM][:, _PERM]  # memory-order gate: G[s_out, s_in, b]
    # reference computes h = t / 10 in float32
    h = (t.astype(np.float32) / np.float32(10)).astype(np.float64)
    I4 = np.eye(S, dtype=np.complex128)
    W = np.zeros((P, P), np.float64)
    for b in range(B):
        M = (-1j) * h[b] * G[:, :, b]
        R = I4 + M + M @ M / 2 + M @ M @ M / 6 + M @ M @ M @ M / 24
        E = np.linalg.matrix_power(R, 10)
        W[b * 8:(b + 1) * 8, b * 8:(b + 1) * 8] = np.block(
            [[E.real, -E.imag], [E.imag, E.real]]
        )
    return W.astype(np.float32)


def _run(inputs, trace=False, trace_cores=None):
    from concourse.bass_utils import run_bass_kernel_spmd

    W = _build_weight(inputs["H_re"], inputs["H_im"], inputs["t"])
    lhsT = np.ascontiguousarray(W.T)  # matmul computes lhsT.T @ rhs

    # Repack state into [p, x] with p = b*8 + c*4 + s.
    sr = np.asarray(inputs["state_re"], np.float32).reshape(X18, S, B)
    si = np.asarray(inputs["state_im"], np.float32).reshape(X18, S, B)
    A = np.empty((B, 2, S, X18), np.float32)
    A[:, 0] = sr.transpose(2, 1, 0)
    A[:, 1] = si.transpose(2, 1, 0)
    A = A.reshape(P, X18)

    in_maps = [
        {"w": lhsT, "x": np.ascontiguousarray(A[:, c * XC:(c + 1) * XC])}
        for c in range(NCORES)
    ]

    nc = _get_nc()
    res = run_bass_kernel_spmd(
        nc,
        in_maps,
        list(range(NCORES)),
        trace=trace,
        trace_cores=trace_cores,
    )

    Y = np.empty((P, X18), np.float32)
    for c in range(NCORES):
        Y[:, c * XC:(c + 1) * XC] = res.results[c]["y"]

    y4 = Y.reshape(B, 2, S, X18)
    out_shape = (2,) * 20 + (B,)
    out = np.empty((2,) + out_shape, np.float32)
    out[0] = y4[:, 0].transpose(2, 1, 0).reshape(out_shape)
    out[1] = y4[:, 1].transpose(2, 1, 0).reshape(out_shape)
    return out, res.exec_time_ns


def kernel(**inputs):
    out, _ = _run(inputs, trace=False)
    return out
